# revision 1
# baseline (speedup 1.0000x reference)
"""CirConv2d kernel for 8 Trainium2 NeuronCores.

Strategy: data-parallel over batch (2 images per core). The circulant
weight synthesis (softmax-mixed block-circulant projections, ~2.25 MB)
is computed on host in numpy (it is 1.5% of the FLOPs); the 3x3 conv —
the dominant cost — runs on device as 9-tap PSUM-accumulated matmuls
over input-channel tiles, using float32r matmuls (full-rate fp32 path
on the PE for moving dim >= 256).
"""

import sys
import numpy as np

sys.path.insert(0, "/opt/trn_rl_repo")

N_CORES = 8
B, C, H = 16, 256, 56
O, I, KS = 256, 256, 3
BPC = B // N_CORES  # batches per core
SEARCH_SPACE = [1, 2, 4, 8, 16, 32, 64]
GUMBEL_SCALE = 1e-4
TAU = 1.0

HP = H + 2            # padded width 58
NPIX = HP * HP        # 3364
ROWS_PER_CHUNK = 8
NCHUNK = H // ROWS_PER_CHUNK  # 7
NCOL = ROWS_PER_CHUNK * H     # 448 output pixels per matmul

_CACHE = {}


def _synth_weight_host(weight, alphas_after):
    w = alphas_after[0] * weight
    for idx, b in enumerate(SEARCH_SPACE[1:], start=1):
        q, p = O // b, I // b
        tmp = weight.reshape(q, b, p, b, KS, KS).transpose(0, 2, 1, 3, 4, 5)
        ii = np.arange(b)[:, None]
        jj = np.arange(b)[None, :]
        rot = tmp[:, :, ii, (ii + jj) % b]          # q,p,b,b,k,k
        cir = rot.mean(axis=2, dtype=np.float32)     # q,p,b,k,k
        out = cir[:, :, (jj - ii) % b]               # q,p,b,b,k,k
        out = out.transpose(0, 2, 1, 3, 4, 5).reshape(O, I, KS, KS)
        w = w + alphas_after[idx] * out
    return w.astype(np.float32)


def _build(reps_dyn=0):
    import concourse.bacc as bacc
    import concourse.bass as bass
    import concourse.mybir as mybir
    from concourse.tile import TileContext

    AP = bass.AP
    f32 = mybir.dt.float32
    f32r = mybir.dt.float32r

    nc = bacc.Bacc("TRN2", target_bir_lowering=False, debug=False,
                   num_devices=N_CORES)
    xin = nc.declare_dram_parameter("x", [BPC, C, H, H], f32, isOutput=False)
    win = nc.declare_dram_parameter("wsynT", [I, O * 9], f32, isOutput=False)
    yout = nc.declare_dram_parameter("y", [BPC, O, H, H], f32, isOutput=True)

    with TileContext(nc) as tc:
        with tc.tile_pool(name="persist", bufs=1) as pp, \
             tc.tile_pool(name="psum", bufs=4, space="PSUM") as psp, \
             tc.tile_pool(name="load", bufs=2) as ldp, \
             tc.tile_pool(name="stage", bufs=4) as stp:
            # small zero tile used to zero the f32r pad borders
            zt = pp.tile([128, 2 * HP], f32, tag="zt")
            nc.vector.memset(zt[:], 0.0)
            # synthesized weight, transposed: [i, o*9+tap], rounded to f32r
            wt = []
            for it in range(2):
                ws = ldp.tile([128, O * 9], f32, tag="wstage")
                nc.sync.dma_start(out=ws[:], in_=win[it * 128:(it + 1) * 128, :])
                t = pp.tile([128, O * 9], f32r, tag=f"w{it}")
                nc.vector.tensor_copy(t[:], ws[:])
                wt.append(t)
            # zero-padded input images: [b][it] -> [128, 58*58] f32r.
            # x DMA lands contiguous; the pad placement + f32r rounding happen
            # in one DVE copy; borders are zeroed from the f32 zero tile.
            xp = [[None] * 2 for _ in range(BPC)]
            for b in range(BPC):
                for it in range(2):
                    t = pp.tile([128, NPIX], f32r, tag=f"xp{b}{it}")
                    ta = t[:]
                    nc.vector.tensor_copy(
                        AP(ta.tensor, ta.offset, [[NPIX, 128], [1, HP]]),
                        zt[:, 0:HP])
                    nc.vector.tensor_copy(
                        AP(ta.tensor, ta.offset + (HP - 1) * HP,
                           [[NPIX, 128], [1, HP]]),
                        zt[:, 0:HP])
                    nc.vector.tensor_copy(
                        AP(ta.tensor, ta.offset, [[NPIX, 128], [HP, HP], [HP - 1, 2]]),
                        zt[:, 0:2 * HP])
                    xs = ldp.tile([128, H * H], f32, tag="xstage")
                    nc.sync.dma_start(out=xs[:], in_=xin[b, it * 128:(it + 1) * 128, :, :])
                    dst = AP(ta.tensor, ta.offset + HP + 1,
                             [[NPIX, 128], [HP, H], [1, H]])
                    nc.vector.tensor_copy(dst, xs[:])
                    xp[b][it] = t
            def conv_body():
                for b in range(BPC):
                    for ot in range(2):
                        for ch in range(NCHUNK):
                            ps = psp.tile([128, NCOL], f32, tag="ps")
                            idx = 0
                            for it in range(2):
                                wap = wt[it][:]
                                for kh in range(3):
                                    for kw in range(3):
                                        t = kh * 3 + kw
                                        lhsT = AP(wap.tensor,
                                                  wap.offset + ot * 128 * 9 + t,
                                                  [[O * 9, 128], [9, 128]])
                                        xap = xp[b][it][:]
                                        rhs = AP(xap.tensor,
                                                 xap.offset + (ch * ROWS_PER_CHUNK + kh) * HP + kw,
                                                 [[NPIX, 128], [HP, ROWS_PER_CHUNK], [1, H]])
                                        nc.tensor.matmul(ps[:], lhsT, rhs,
                                                         start=(idx == 0),
                                                         stop=(idx == 17))
                                        idx += 1
                            st = stp.tile([128, NCOL], f32, tag="st")
                            nc.scalar.copy(out=st[:], in_=ps[:])
                            ybase = (b * O + ot * 128) * (H * H) + ch * NCOL
                            dst = AP(yout[:].tensor, ybase, [[H * H, 128], [1, NCOL]])
                            nc.sync.dma_start(out=dst, in_=st[:])

            if reps_dyn:
                with tc.For_i(0, reps_dyn, 1):
                    conv_body()
            else:
                conv_body()
    nc.compile()
    return nc


def _get_nc():
    if "nc" not in _CACHE:
        _CACHE["nc"] = _build()
    return _CACHE["nc"]


def _host_prep(x, weight, alphas, gumbels):
    x = np.ascontiguousarray(np.asarray(x, dtype=np.float32))
    weight = np.asarray(weight, dtype=np.float32)
    alphas = np.asarray(alphas, dtype=np.float32)
    gumbels = np.asarray(gumbels, dtype=np.float32)

    a = (alphas + np.float32(GUMBEL_SCALE) * gumbels) / np.float32(TAU)
    a = a - a.max()
    e = np.exp(a, dtype=np.float32)
    alphas_after = (e / e.sum(dtype=np.float32)).astype(np.float32)

    w = _synth_weight_host(weight, alphas_after)  # [O, I, 3, 3]
    wsynT = np.ascontiguousarray(
        w.reshape(O, I, 9).transpose(1, 0, 2).reshape(I, O * 9).astype(np.float32))
    return x, wsynT


def kernel(x, weight, alphas, gumbels):
    x, wsynT = _host_prep(x, weight, alphas, gumbels)
    nc = _get_nc()

    from concourse.bass_utils import run_bass_kernel_spmd
    in_maps = [{"x": x[i * BPC:(i + 1) * BPC], "wsynT": wsynT}
               for i in range(N_CORES)]
    res = run_bass_kernel_spmd(nc, in_maps, list(range(N_CORES)))
    out = np.concatenate([res.results[i]["y"] for i in range(N_CORES)], axis=0)
    return np.ascontiguousarray(out.astype(np.float32))



# revision 2
# speedup vs baseline: 1.1547x; 1.1547x over previous
"""CirConv2d kernel for 8 Trainium2 NeuronCores.

Strategy: data-parallel over batch (2 images per core). The circulant
weight synthesis (softmax-mixed block-circulant projections, ~2.25 MB)
is computed on host in numpy (it is 1.5% of the FLOPs); the 3x3 conv —
the dominant cost — runs on device as 9-tap PSUM-accumulated matmuls
over input-channel tiles.

v2: both matmul operands in bf16. f32r matmuls do a serial in-instruction
4-byte weight load (~107 ns each, not overlappable); bf16 weights get a
separate LDWEIGHTS with FWL (fast weight load) that the PE pulls ahead
into the background weight buffer, hiding it behind the previous matmul's
stream. Weights are stored tap-major ([I, 9, O]) so each 128-column
weight block is contiguous (FWL requires contiguous reads).
"""

import sys
import numpy as np

sys.path.insert(0, "/opt/trn_rl_repo")

N_CORES = 8
B, C, H = 16, 256, 56
O, I, KS = 256, 256, 3
BPC = B // N_CORES  # batches per core
SEARCH_SPACE = [1, 2, 4, 8, 16, 32, 64]
GUMBEL_SCALE = 1e-4
TAU = 1.0

HP = H + 2            # padded width 58
NPIX = HP * HP        # 3364
ROWS_PER_CHUNK = 8
NCHUNK = H // ROWS_PER_CHUNK  # 7
NCOL = ROWS_PER_CHUNK * H     # 448 output pixels per matmul

_CACHE = {}


def _synth_weight_host(weight, alphas_after):
    w = alphas_after[0] * weight
    for idx, b in enumerate(SEARCH_SPACE[1:], start=1):
        q, p = O // b, I // b
        tmp = weight.reshape(q, b, p, b, KS, KS).transpose(0, 2, 1, 3, 4, 5)
        ii = np.arange(b)[:, None]
        jj = np.arange(b)[None, :]
        rot = tmp[:, :, ii, (ii + jj) % b]          # q,p,b,b,k,k
        cir = rot.mean(axis=2, dtype=np.float32)     # q,p,b,k,k
        out = cir[:, :, (jj - ii) % b]               # q,p,b,b,k,k
        out = out.transpose(0, 2, 1, 3, 4, 5).reshape(O, I, KS, KS)
        w = w + alphas_after[idx] * out
    return w.astype(np.float32)


def _build(reps_dyn=0):
    import concourse.bacc as bacc
    import concourse.bass as bass
    import concourse.mybir as mybir
    from concourse.tile import TileContext

    AP = bass.AP
    f32 = mybir.dt.float32
    bf16 = mybir.dt.bfloat16

    nc = bacc.Bacc("TRN2", target_bir_lowering=False, debug=False,
                   num_devices=N_CORES)
    xin = nc.declare_dram_parameter("x", [BPC, C, H, H], f32, isOutput=False)
    win = nc.declare_dram_parameter("wsynT", [I, 9 * O], f32, isOutput=False)
    yout = nc.declare_dram_parameter("y", [BPC, O, H, H], f32, isOutput=True)

    with TileContext(nc) as tc:
        with tc.tile_pool(name="persist", bufs=1) as pp, \
             tc.tile_pool(name="psum", bufs=4, space="PSUM") as psp, \
             tc.tile_pool(name="load", bufs=2) as ldp, \
             tc.tile_pool(name="stage", bufs=4) as stp:
            # small zero tile used to zero the bf16 pad borders
            zt = pp.tile([128, 2 * HP], bf16, tag="zt")
            nc.vector.memset(zt[:], 0.0)
            # synthesized weight, transposed, tap-major: [i, tap*O + ot*128 + oc]
            # so each 128-wide lhsT block is contiguous (FWL-eligible), bf16.
            wt = []
            for it in range(2):
                ws = ldp.tile([128, 9 * O], f32, tag="wstage")
                nc.sync.dma_start(out=ws[:], in_=win[it * 128:(it + 1) * 128, :])
                t = pp.tile([128, 9 * O], bf16, tag=f"w{it}")
                nc.vector.tensor_copy(t[:], ws[:])
                wt.append(t)
            # zero-padded input images: [b][it] -> [128, 58*58] bf16.
            xp = [[None] * 2 for _ in range(BPC)]
            for b in range(BPC):
                for it in range(2):
                    t = pp.tile([128, NPIX], bf16, tag=f"xp{b}{it}")
                    ta = t[:]
                    nc.vector.tensor_copy(
                        AP(ta.tensor, ta.offset, [[NPIX, 128], [1, HP]]),
                        zt[:, 0:HP])
                    nc.vector.tensor_copy(
                        AP(ta.tensor, ta.offset + (HP - 1) * HP,
                           [[NPIX, 128], [1, HP]]),
                        zt[:, 0:HP])
                    nc.vector.tensor_copy(
                        AP(ta.tensor, ta.offset, [[NPIX, 128], [HP, HP], [HP - 1, 2]]),
                        zt[:, 0:2 * HP])
                    xs = ldp.tile([128, H * H], f32, tag="xstage")
                    nc.sync.dma_start(out=xs[:], in_=xin[b, it * 128:(it + 1) * 128, :, :])
                    dst = AP(ta.tensor, ta.offset + HP + 1,
                             [[NPIX, 128], [HP, H], [1, H]])
                    nc.vector.tensor_copy(dst, xs[:])
                    xp[b][it] = t

            def conv_body():
                for b in range(BPC):
                    for ot in range(2):
                        for ch in range(NCHUNK):
                            ps = psp.tile([128, NCOL], f32, tag="ps")
                            idx = 0
                            for it in range(2):
                                wap = wt[it][:]
                                for kh in range(3):
                                    for kw in range(3):
                                        t = kh * 3 + kw
                                        lhsT = AP(wap.tensor,
                                                  wap.offset + t * O + ot * 128,
                                                  [[9 * O, 128], [1, 128]])
                                        xap = xp[b][it][:]
                                        rhs = AP(xap.tensor,
                                                 xap.offset + (ch * ROWS_PER_CHUNK + kh) * HP + kw,
                                                 [[NPIX, 128], [HP, ROWS_PER_CHUNK], [1, H]])
                                        nc.tensor.matmul(ps[:], lhsT, rhs,
                                                         start=(idx == 0),
                                                         stop=(idx == 17))
                                        idx += 1
                            st = stp.tile([128, NCOL], f32, tag="st")
                            nc.scalar.copy(out=st[:], in_=ps[:])
                            ybase = (b * O + ot * 128) * (H * H) + ch * NCOL
                            dst = AP(yout[:].tensor, ybase, [[H * H, 128], [1, NCOL]])
                            nc.sync.dma_start(out=dst, in_=st[:])

            if reps_dyn:
                with tc.For_i(0, reps_dyn, 1):
                    conv_body()
            else:
                conv_body()
    nc.compile()
    return nc


def _get_nc():
    if "nc" not in _CACHE:
        _CACHE["nc"] = _build()
    return _CACHE["nc"]


def _host_prep(x, weight, alphas, gumbels):
    x = np.ascontiguousarray(np.asarray(x, dtype=np.float32))
    weight = np.asarray(weight, dtype=np.float32)
    alphas = np.asarray(alphas, dtype=np.float32)
    gumbels = np.asarray(gumbels, dtype=np.float32)

    a = (alphas + np.float32(GUMBEL_SCALE) * gumbels) / np.float32(TAU)
    a = a - a.max()
    e = np.exp(a, dtype=np.float32)
    alphas_after = (e / e.sum(dtype=np.float32)).astype(np.float32)

    w = _synth_weight_host(weight, alphas_after)  # [O, I, 3, 3]
    # [I, 9, O]: tap-major so each [128-cin, 128-cout] lhsT block is contiguous
    wsynT = np.ascontiguousarray(
        w.reshape(O, I, 9).transpose(1, 2, 0).reshape(I, 9 * O).astype(np.float32))
    return x, wsynT


def kernel(x, weight, alphas, gumbels):
    x, wsynT = _host_prep(x, weight, alphas, gumbels)
    nc = _get_nc()

    from concourse.bass_utils import run_bass_kernel_spmd
    in_maps = [{"x": x[i * BPC:(i + 1) * BPC], "wsynT": wsynT}
               for i in range(N_CORES)]
    res = run_bass_kernel_spmd(nc, in_maps, list(range(N_CORES)))
    out = np.concatenate([res.results[i]["y"] for i in range(N_CORES)], axis=0)
    return np.ascontiguousarray(out.astype(np.float32))


# revision 5
# speedup vs baseline: 1.4824x; 1.2838x over previous
"""CirConv2d kernel for 8 Trainium2 NeuronCores — 1D Winograd F(2,3).

Data-parallel over batch (2 images per core). Weight synthesis on host.
The 3x3 conv runs as Winograd F(2,3) along the W axis: 4 transform-domain
"taps" j replace the 3 horizontal taps at half the output width, cutting
PE cycles 1.5x vs direct (2 outputs cost 4 multiplies instead of 6).
Vertical taps stay direct (3 kh offsets into the V buffers).

  V_j = B^T d per 4-wide window (stride 2):   V0=d0-d2, V1=d1+d2,
        V2=d2-d1, V3=d1-d3            (GpSimd, inside the loop)
  M_j[cout, r, t] = sum_{cin,kh} Gw[j,kh][cout,cin] V_j[cin, r+kh, t]
                                        (PE: 6 accumulating bf16 matmuls)
  out[r, 2t]   = M0 + M1 + M2          (DVE from PSUM)
  out[r, 2t+1] = M1 - M2 - M3

bf16 operands: weights get LDWEIGHTS+FWL hidden behind the previous
matmul's stream (f32r pays a serial ~107ns in-instruction weight load).
"""

import sys
import numpy as np

sys.path.insert(0, "/opt/trn_rl_repo")

N_CORES = 8
B, C, H = 16, 256, 56
O, I, KS = 256, 256, 3
BPC = B // N_CORES  # batches per core
SEARCH_SPACE = [1, 2, 4, 8, 16, 32, 64]
GUMBEL_SCALE = 1e-4
TAU = 1.0

HP = H + 2            # padded width 58
NPIX = HP * HP        # 3364
NT = H // 2           # 28 Winograd tiles per row
NV = HP * NT          # 1624 V elements per partition per j
# output row chunks: 3x16 + 1x8 rows; moving dim = nr*28 (<=448, one PSUM bank)
CHUNKS = [(0, 16), (16, 16), (32, 16), (48, 8)]

_CACHE = {}


def _synth_weight_host(weight, alphas_after):
    w = alphas_after[0] * weight
    for idx, b in enumerate(SEARCH_SPACE[1:], start=1):
        q, p = O // b, I // b
        tmp = weight.reshape(q, b, p, b, KS, KS).transpose(0, 2, 1, 3, 4, 5)
        ii = np.arange(b)[:, None]
        jj = np.arange(b)[None, :]
        rot = tmp[:, :, ii, (ii + jj) % b]          # q,p,b,b,k,k
        cir = rot.mean(axis=2, dtype=np.float32)     # q,p,b,k,k
        out = cir[:, :, (jj - ii) % b]               # q,p,b,b,k,k
        out = out.transpose(0, 2, 1, 3, 4, 5).reshape(O, I, KS, KS)
        w = w + alphas_after[idx] * out
    return w.astype(np.float32)


def _build(reps_dyn=0):
    import concourse.bacc as bacc
    import concourse.bass as bass
    import concourse.mybir as mybir
    from concourse.tile import TileContext

    AP = bass.AP
    f32 = mybir.dt.float32
    bf16 = mybir.dt.bfloat16
    alu = mybir.AluOpType

    nc = bacc.Bacc("TRN2", target_bir_lowering=False, debug=False,
                   num_devices=N_CORES)
    xin = nc.declare_dram_parameter("x", [BPC, C, H, H], bf16, isOutput=False)
    # Winograd-domain weights, [I, j(4) * kh(3) * ot(2) * 128], bf16 from host
    win = nc.declare_dram_parameter("wsynT", [I, 4 * 3 * O], bf16, isOutput=False)
    yout = nc.declare_dram_parameter("y", [BPC, O, H, H], f32, isOutput=True)

    with TileContext(nc) as tc:
        with tc.tile_pool(name="persist", bufs=1) as pp, \
             tc.tile_pool(name="psum", bufs=2, space="PSUM") as psp, \
             tc.tile_pool(name="vbuf", bufs=2) as vp, \
             tc.tile_pool(name="stage", bufs=4) as stp:
            zt = pp.tile([128, 2 * HP], bf16, tag="zt")
            nc.vector.memset(zt[:], 0.0)
            # transform-domain weights: contiguous 128-col blocks per
            # (j, kh, ot) -> FWL-eligible bf16 LDWEIGHTS
            wt = []
            for it in range(2):
                t = pp.tile([128, 12 * O], bf16, tag=f"w{it}")
                nc.sync.dma_start(out=t[:], in_=win[it * 128:(it + 1) * 128, :])
                wt.append(t)
            # zero-padded input images: [b][it] -> [128, 58*58] bf16;
            # interior DMAed straight from HBM (bf16), borders zeroed by DVE
            xp = [[None] * 2 for _ in range(BPC)]
            for b in range(BPC):
                for it in range(2):
                    t = pp.tile([128, NPIX], bf16, tag=f"xp{b}{it}")
                    ta = t[:]
                    nc.vector.tensor_copy(
                        AP(ta.tensor, ta.offset, [[NPIX, 128], [1, HP]]),
                        zt[:, 0:HP])
                    nc.vector.tensor_copy(
                        AP(ta.tensor, ta.offset + (HP - 1) * HP,
                           [[NPIX, 128], [1, HP]]),
                        zt[:, 0:HP])
                    nc.vector.tensor_copy(
                        AP(ta.tensor, ta.offset, [[NPIX, 128], [HP, HP], [HP - 1, 2]]),
                        zt[:, 0:2 * HP])
                    dst = AP(ta.tensor, ta.offset + HP + 1,
                             [[NPIX, 128], [HP, H], [1, H]])
                    nc.sync.dma_start(out=dst, in_=xin[b, it * 128:(it + 1) * 128, :, :])
                    xp[b][it] = t

            # (in0 shift, in1 shift, alu op) per j: V_j from xpad cols 2t+shift
            VDEFS = [(0, 2, alu.subtract), (1, 2, alu.add),
                     (2, 1, alu.subtract), (1, 3, alu.subtract)]

            def body():
                # input transform on GpSimd (no PSUM access needed), bf16 out
                vt = [[[None] * 4 for _ in range(2)] for _ in range(BPC)]
                for b in range(BPC):
                    for it in range(2):
                        xa = xp[b][it][:]
                        for j, (s0, s1, op) in enumerate(VDEFS):
                            v = vp.tile([128, NV], bf16, tag=f"v{b}{it}{j}")
                            va = v[:]
                            nc.gpsimd.tensor_tensor(
                                AP(va.tensor, va.offset, [[NV, 128], [1, NV]]),
                                AP(xa.tensor, xa.offset + s0,
                                   [[NPIX, 128], [HP, HP], [2, NT]]),
                                AP(xa.tensor, xa.offset + s1,
                                   [[NPIX, 128], [HP, HP], [2, NT]]),
                                op)
                            vt[b][it][j] = v
                # Winograd-domain matmuls + output transform
                for b in range(BPC):
                    for ot in range(2):
                        for (r0, nr) in CHUNKS:
                            ncol = nr * NT
                            ms = []
                            for j in range(4):
                                ps = psp.tile([128, 448], f32, tag=f"m{j}")
                                idx = 0
                                for it in range(2):
                                    wa = wt[it][:]
                                    for kh in range(3):
                                        lhsT = AP(wa.tensor,
                                                  wa.offset + ((j * 3 + kh) * 2 + ot) * 128,
                                                  [[12 * O, 128], [1, 128]])
                                        va = vt[b][it][j][:]
                                        rhs = AP(va.tensor,
                                                 va.offset + (r0 + kh) * NT,
                                                 [[NV, 128], [1, ncol]])
                                        nc.tensor.matmul(ps[:, 0:ncol], lhsT, rhs,
                                                         start=(idx == 0),
                                                         stop=(idx == 5))
                                        idx += 1
                                ms.append(ps)
                            # output transform: even = M0+M1+M2, odd = M1-M2-M3.
                            # DVE may read only ONE PSUM operand per op, so
                            # M1 is first copied to SBUF on the scalar engine.
                            c1 = stp.tile([128, 448], f32, tag="c1")
                            t0 = stp.tile([128, 448], f32, tag="t0")
                            t1 = stp.tile([128, 448], f32, tag="t1")
                            yst = stp.tile([128, nr * H], f32, tag="yst")
                            ya = yst[:]
                            nc.scalar.copy(out=c1[:, 0:ncol], in_=ms[1][:, 0:ncol])
                            nc.vector.tensor_add(t0[:, 0:ncol], c1[:, 0:ncol],
                                                 ms[0][:, 0:ncol])
                            nc.vector.tensor_sub(t1[:, 0:ncol], c1[:, 0:ncol],
                                                 ms[2][:, 0:ncol])
                            nc.vector.tensor_add(
                                AP(ya.tensor, ya.offset, [[nr * H, 128], [H, nr], [2, NT]]),
                                t0[:, 0:ncol], ms[2][:, 0:ncol])
                            nc.vector.tensor_sub(
                                AP(ya.tensor, ya.offset + 1, [[nr * H, 128], [H, nr], [2, NT]]),
                                t1[:, 0:ncol], ms[3][:, 0:ncol])
                            ybase = (b * O + ot * 128) * (H * H) + r0 * H
                            dst = AP(yout[:].tensor, ybase, [[H * H, 128], [1, nr * H]])
                            nc.sync.dma_start(out=dst, in_=yst[:, 0:nr * H])

            if reps_dyn:
                with tc.For_i(0, reps_dyn, 1):
                    body()
            else:
                body()
    nc.compile()
    return nc


def _get_nc():
    if "nc" not in _CACHE:
        _CACHE["nc"] = _build()
    return _CACHE["nc"]


def _host_prep(x, weight, alphas, gumbels):
    x = np.ascontiguousarray(np.asarray(x, dtype=np.float32))
    weight = np.asarray(weight, dtype=np.float32)
    alphas = np.asarray(alphas, dtype=np.float32)
    gumbels = np.asarray(gumbels, dtype=np.float32)

    a = (alphas + np.float32(GUMBEL_SCALE) * gumbels) / np.float32(TAU)
    a = a - a.max()
    e = np.exp(a, dtype=np.float32)
    alphas_after = (e / e.sum(dtype=np.float32)).astype(np.float32)

    w = _synth_weight_host(weight, alphas_after)  # [O, I, 3, 3]
    # Winograd weight transform along kw: Gw[j,o,i,kh] = sum_kw G[j,kw] w[o,i,kh,kw]
    G = np.array([[1, 0, 0], [0.5, 0.5, 0.5], [0.5, -0.5, 0.5], [0, 0, 1]],
                 dtype=np.float32)
    Gw = np.einsum('jw,oihw->joih', G, w).astype(np.float32)  # [4, O, I, 3]
    # lhsT layout [i, j, kh, ot, oc]
    lhsT = Gw.transpose(2, 0, 3, 1).reshape(I, 4, 3, 2, 128)
    import ml_dtypes
    bf16 = ml_dtypes.bfloat16
    wsynT = np.ascontiguousarray(lhsT.reshape(I, 12 * O).astype(bf16))
    return np.ascontiguousarray(x.astype(bf16)), wsynT


def kernel(x, weight, alphas, gumbels):
    x, wsynT = _host_prep(x, weight, alphas, gumbels)
    nc = _get_nc()

    from concourse.bass_utils import run_bass_kernel_spmd
    in_maps = [{"x": x[i * BPC:(i + 1) * BPC], "wsynT": wsynT}
               for i in range(N_CORES)]
    res = run_bass_kernel_spmd(nc, in_maps, list(range(N_CORES)))
    out = np.concatenate([res.results[i]["y"] for i in range(N_CORES)], axis=0)
    return np.ascontiguousarray(out.astype(np.float32))


# revision 11
# speedup vs baseline: 1.4855x; 1.0021x over previous
"""CirConv2d kernel for 8 Trainium2 NeuronCores — 1D Winograd F(2,3).

Data-parallel over batch (2 images per core). Weight synthesis on host.
The 3x3 conv runs as Winograd F(2,3) along the W axis: 4 transform-domain
"taps" j replace the 3 horizontal taps at half the output width, cutting
PE cycles 1.5x vs direct (2 outputs cost 4 multiplies instead of 6).
Vertical taps stay direct (3 kh offsets into the V buffers).

  V_j = B^T d per 4-wide window (stride 2):   V0=d0-d2, V1=d1+d2,
        V2=d2-d1, V3=d1-d3            (GpSimd, inside the loop)
  M_j[cout, r, t] = sum_{cin,kh} Gw[j,kh][cout,cin] V_j[cin, r+kh, t]
                                        (PE: 6 accumulating bf16 matmuls)
  out[r, 2t]   = M0 + M1 + M2          (DVE from PSUM)
  out[r, 2t+1] = M1 - M2 - M3

bf16 operands: weights get LDWEIGHTS+FWL hidden behind the previous
matmul's stream (f32r pays a serial ~107ns in-instruction weight load).
"""

import sys
import numpy as np

sys.path.insert(0, "/opt/trn_rl_repo")

N_CORES = 8
B, C, H = 16, 256, 56
O, I, KS = 256, 256, 3
BPC = B // N_CORES  # batches per core
SEARCH_SPACE = [1, 2, 4, 8, 16, 32, 64]
GUMBEL_SCALE = 1e-4
TAU = 1.0

HP = H + 2            # padded width 58
NPIX = HP * HP        # 3364
NT = H // 2           # 28 Winograd tiles per row
NV = HP * NT          # 1624 V elements per partition per j
# output row chunks: 3x16 + 1x8 rows; moving dim = nr*28 (<=448, one PSUM bank)
CHUNKS = [(0, 16), (16, 16), (32, 16), (48, 8)]

_CACHE = {}


def _synth_weight_host(weight, alphas_after):
    w = alphas_after[0] * weight
    for idx, b in enumerate(SEARCH_SPACE[1:], start=1):
        q, p = O // b, I // b
        tmp = weight.reshape(q, b, p, b, KS, KS).transpose(0, 2, 1, 3, 4, 5)
        ii = np.arange(b)[:, None]
        jj = np.arange(b)[None, :]
        rot = tmp[:, :, ii, (ii + jj) % b]          # q,p,b,b,k,k
        cir = rot.mean(axis=2, dtype=np.float32)     # q,p,b,k,k
        out = cir[:, :, (jj - ii) % b]               # q,p,b,b,k,k
        out = out.transpose(0, 2, 1, 3, 4, 5).reshape(O, I, KS, KS)
        w = w + alphas_after[idx] * out
    return w.astype(np.float32)


def _build(reps_dyn=0):
    import concourse.bacc as bacc
    import concourse.bass as bass
    import concourse.mybir as mybir
    from concourse.tile import TileContext

    AP = bass.AP
    f32 = mybir.dt.float32
    bf16 = mybir.dt.bfloat16
    alu = mybir.AluOpType

    nc = bacc.Bacc("TRN2", target_bir_lowering=False, debug=False,
                   num_devices=N_CORES)
    xin = nc.declare_dram_parameter("x", [BPC, C, H, H], bf16, isOutput=False)
    # Winograd-domain weights, [I, j(4) * kh(3) * ot(2) * 128], bf16 from host
    win = nc.declare_dram_parameter("wsynT", [I, 4 * 3 * O], bf16, isOutput=False)
    yout = nc.declare_dram_parameter("y", [BPC, O, H, H], f32, isOutput=True)

    with TileContext(nc) as tc:
        with tc.tile_pool(name="persist", bufs=1) as pp, \
             tc.tile_pool(name="psum", bufs=2, space="PSUM") as psp, \
             tc.tile_pool(name="vbuf", bufs=2) as vp, \
             tc.tile_pool(name="stage", bufs=4) as stp:
            zt = pp.tile([128, 2 * HP], bf16, tag="zt")
            nc.vector.memset(zt[:], 0.0)
            # transform-domain weights: contiguous 128-col blocks per
            # (j, kh, ot) -> FWL-eligible bf16 LDWEIGHTS
            wt = []
            for it in range(2):
                t = pp.tile([128, 12 * O], bf16, tag=f"w{it}")
                nc.sync.dma_start(out=t[:], in_=win[it * 128:(it + 1) * 128, :])
                wt.append(t)
            # zero-padded input images: [b][it] -> [128, 58*58] bf16;
            # interior DMAed straight from HBM (bf16), borders zeroed by DVE
            xp = [[None] * 2 for _ in range(BPC)]
            for b in range(BPC):
                for it in range(2):
                    t = pp.tile([128, NPIX], bf16, tag=f"xp{b}{it}")
                    ta = t[:]
                    nc.vector.tensor_copy(
                        AP(ta.tensor, ta.offset, [[NPIX, 128], [1, HP]]),
                        zt[:, 0:HP])
                    nc.vector.tensor_copy(
                        AP(ta.tensor, ta.offset + (HP - 1) * HP,
                           [[NPIX, 128], [1, HP]]),
                        zt[:, 0:HP])
                    nc.vector.tensor_copy(
                        AP(ta.tensor, ta.offset, [[NPIX, 128], [HP, HP], [HP - 1, 2]]),
                        zt[:, 0:2 * HP])
                    dst = AP(ta.tensor, ta.offset + HP + 1,
                             [[NPIX, 128], [HP, H], [1, H]])
                    nc.sync.dma_start(out=dst, in_=xin[b, it * 128:(it + 1) * 128, :, :])
                    xp[b][it] = t

            # (in0 shift, in1 shift, alu op) per j: V_j from xpad cols 2t+shift
            VDEFS = [(0, 2, alu.subtract), (1, 2, alu.add),
                     (2, 1, alu.subtract), (1, 3, alu.subtract)]

            def body():
                # input transform on GpSimd (no PSUM access needed), bf16 out
                vt = [[[None] * 4 for _ in range(2)] for _ in range(BPC)]
                for b in range(BPC):
                    for it in range(2):
                        xa = xp[b][it][:]
                        for j, (s0, s1, op) in enumerate(VDEFS):
                            v = vp.tile([128, NV], bf16, tag=f"v{b}{it}{j}")
                            va = v[:]
                            nc.gpsimd.tensor_tensor(
                                AP(va.tensor, va.offset, [[NV, 128], [1, NV]]),
                                AP(xa.tensor, xa.offset + s0,
                                   [[NPIX, 128], [HP, HP], [2, NT]]),
                                AP(xa.tensor, xa.offset + s1,
                                   [[NPIX, 128], [HP, HP], [2, NT]]),
                                op)
                            vt[b][it][j] = v
                # Winograd-domain matmuls + output transform
                for b in range(BPC):
                    for ot in range(2):
                        for (r0, nr) in CHUNKS:
                            ncol = nr * NT
                            ms = []
                            for j in range(4):
                                ps = psp.tile([128, 448], f32, tag=f"m{j}")
                                idx = 0
                                for it in range(2):
                                    wa = wt[it][:]
                                    for kh in range(3):
                                        lhsT = AP(wa.tensor,
                                                  wa.offset + ((j * 3 + kh) * 2 + ot) * 128,
                                                  [[12 * O, 128], [1, 128]])
                                        va = vt[b][it][j][:]
                                        rhs = AP(va.tensor,
                                                 va.offset + (r0 + kh) * NT,
                                                 [[NV, 128], [1, ncol]])
                                        nc.tensor.matmul(ps[:, 0:ncol], lhsT, rhs,
                                                         start=(idx == 0),
                                                         stop=(idx == 5))
                                        idx += 1
                                ms.append(ps)
                            # output transform: even = M0+M1+M2, odd = M1-M2-M3.
                            # DVE may read only ONE PSUM operand per op, so
                            # M1 is first copied to SBUF on the scalar engine.
                            c1 = stp.tile([128, 448], f32, tag="c1")
                            t0 = stp.tile([128, 448], f32, tag="t0")
                            t1 = stp.tile([128, 448], f32, tag="t1")
                            yst = stp.tile([128, nr * H], f32, tag="yst")
                            ya = yst[:]
                            nc.scalar.copy(out=c1[:, 0:ncol], in_=ms[1][:, 0:ncol])
                            nc.vector.tensor_add(t0[:, 0:ncol], c1[:, 0:ncol],
                                                 ms[0][:, 0:ncol])
                            nc.vector.tensor_sub(t1[:, 0:ncol], c1[:, 0:ncol],
                                                 ms[2][:, 0:ncol])
                            nc.vector.tensor_add(
                                AP(ya.tensor, ya.offset, [[nr * H, 128], [H, nr], [2, NT]]),
                                t0[:, 0:ncol], ms[2][:, 0:ncol])
                            nc.vector.tensor_sub(
                                AP(ya.tensor, ya.offset + 1, [[nr * H, 128], [H, nr], [2, NT]]),
                                t1[:, 0:ncol], ms[3][:, 0:ncol])
                            ybase = (b * O + ot * 128) * (H * H) + r0 * H
                            dst = AP(yout[:].tensor, ybase, [[H * H, 128], [1, nr * H]])
                            nc.sync.dma_start(out=dst, in_=yst[:, 0:nr * H])

            if reps_dyn:
                with tc.For_i(0, reps_dyn, 1):
                    body()
            else:
                body()
    nc.compile()
    return nc


def _get_nc():
    if "nc" not in _CACHE:
        _CACHE["nc"] = _build()
    return _CACHE["nc"]


def _host_prep(x, weight, alphas, gumbels):
    x = np.ascontiguousarray(np.asarray(x, dtype=np.float32))
    weight = np.asarray(weight, dtype=np.float32)
    alphas = np.asarray(alphas, dtype=np.float32)
    gumbels = np.asarray(gumbels, dtype=np.float32)

    a = (alphas + np.float32(GUMBEL_SCALE) * gumbels) / np.float32(TAU)
    a = a - a.max()
    e = np.exp(a, dtype=np.float32)
    alphas_after = (e / e.sum(dtype=np.float32)).astype(np.float32)

    w = _synth_weight_host(weight, alphas_after)  # [O, I, 3, 3]
    # Winograd weight transform along kw: Gw[j,o,i,kh] = sum_kw G[j,kw] w[o,i,kh,kw]
    G = np.array([[1, 0, 0], [0.5, 0.5, 0.5], [0.5, -0.5, 0.5], [0, 0, 1]],
                 dtype=np.float32)
    Gw = np.einsum('jw,oihw->joih', G, w).astype(np.float32)  # [4, O, I, 3]
    # lhsT layout [i, j, kh, ot, oc]
    lhsT = Gw.transpose(2, 0, 3, 1).reshape(I, 4, 3, 2, 128)
    import ml_dtypes
    bf16 = ml_dtypes.bfloat16
    wsynT = np.ascontiguousarray(lhsT.reshape(I, 12 * O).astype(bf16))
    return np.ascontiguousarray(x.astype(bf16)), wsynT


def kernel(x, weight, alphas, gumbels):
    x, wsynT = _host_prep(x, weight, alphas, gumbels)
    nc = _get_nc()

    from concourse.bass_utils import run_bass_kernel_spmd
    in_maps = [{"x": x[i * BPC:(i + 1) * BPC], "wsynT": wsynT}
               for i in range(N_CORES)]
    res = run_bass_kernel_spmd(nc, in_maps, list(range(N_CORES)))
    out = np.concatenate([res.results[i]["y"] for i in range(N_CORES)], axis=0)
    return np.ascontiguousarray(out.astype(np.float32))
